# revision 1
# baseline (speedup 1.0000x reference)
"""MetaQuickSR Trainium2 kernel (8-core SPMD, row-sharded).

Sharding: H=256 output-feature rows split 32/core (+4-row conv halo).
Each core computes: 4-layer CNN -> implicit im2col -> Pos2Weight MLP ->
per-pixel locally-connected matmul -> its 64-row slab of the (4,3,512,512)
output.  No cross-core communication.
"""

import numpy as np
import ml_dtypes

import concourse.bass as bass
import concourse.mybir as mybir
from concourse.tile import TileContext
from concourse.bass_utils import run_bass_kernel_spmd
from concourse.dve_ops import TENSOR_TENSOR_REDUCE

BF16 = ml_dtypes.bfloat16

NCORES = 8
N, CI, Himg, Wimg, S = 4, 16, 256, 256, 2
ROWS = Himg // NCORES          # 32 output-feature rows per core
HALO = 4
NR = ROWS + 2 * HALO           # 40 buffered rows
WP = Wimg + 2                  # 258 zero-padded width
NPIX = ROWS * Wimg             # 8192 einsum pixels per core
NT = NPIX // 128               # 64 pixel tiles
PCH = 8                        # 1024-pixel chunks per q plane
RGB_MEAN = (0.4488, 0.4371, 0.404)
RGB_RANGE = 255.0

_NC = None


def _legalize_waits(nc, lim=1):
    """This walrus build accepts only one sync-wait per instruction; move
    surplus waits onto same-engine NoOps inserted just before."""
    cnt = 0
    for f in nc.m.functions:
        for bb in f.blocks:
            new = []
            for inst in bb.instructions:
                si = inst.sync_info
                if si is not None and si.on_wait is not None \
                        and len(si.on_wait) > lim:
                    waits = list(si.on_wait)
                    excess, keep = waits[:-lim], waits[-lim:]
                    for w in excess:
                        cnt += 1
                        nop = mybir.InstNoOp(
                            name=f"I-lw{cnt}", opcode="NoOp",
                            engine=inst.engine, debug=inst.debug,
                            ins=[], outs=[],
                            sync_info=mybir.SyncInfo(on_wait=[w],
                                                     on_update=[]))
                        new.append(nop)
                        nc.inst_map[nop.name] = nop
                    inst.sync_info = mybir.SyncInfo(
                        on_wait=keep, on_update=list(si.on_update or []))
                new.append(inst)
            bb.instructions = new
    return cnt


def _build_program():
    nc = bass.Bass(trn_type="TRN2")
    f32 = mybir.dt.float32
    bf = mybir.dt.bfloat16

    # packed constant inputs: [x | cw | w2p] bf16,
    # [w1 | cb | b1c | b2p | ones | mean-shift] f32
    BFW = NR * WP + 4 * 9 * 16 + 2 * 432          # 11760
    FW = 256 + 4 + 2 + 432 + 128 + NT * 12        # 1590
    bfin = nc.dram_tensor("bfin", [128, BFW], bf, kind="ExternalInput")
    f32in = nc.dram_tensor("f32in", [128, FW], f32, kind="ExternalInput")
    post = nc.dram_tensor("post", [4, 3, NPIX], f32, kind="ExternalInput")
    outd = nc.dram_tensor("out", [4, 3, 2 * ROWS, 2 * Wimg], f32,
                          kind="ExternalOutput")

    with TileContext(nc) as tc:
        with (
            tc.tile_pool(name="singles", bufs=1) as singles,
            tc.tile_pool(name="pos_p", bufs=2) as pos_p,
            tc.tile_pool(name="ht_p", bufs=2) as ht_p,
            tc.tile_pool(name="lws_p", bufs=3) as lws_p,
            tc.tile_pool(name="scr_p", bufs=2) as scr_p,
            tc.tile_pool(name="cps", bufs=2, space="PSUM") as cps,
            tc.tile_pool(name="hps", bufs=2, space="PSUM") as hps,
            tc.tile_pool(name="lps", bufs=2, space="PSUM") as lps,
        ):
            # ---- resident inputs -------------------------------------
            bf_sb = singles.tile([128, BFW], bf)
            f32_sb = singles.tile([128, FW], f32)
            fA = singles.tile([128, NR, WP], bf)
            fB = singles.tile([128, NR, WP], bf)
            f4c = singles.tile([64, NR, WP], bf)
            # fT2h[half][p, (row,kw), (n,ci)]: transposed f4 rows 3..36 with
            # 3 horizontal shifts; a tile's 9 tap blocks are equally spaced
            # (tap stride 64) so one image's patch is a 2-free-dim AP.
            fT2h = [singles.tile([128, 34 * 3 * 64], bf, name=f"fT2h{h}")
                    for h in range(2)]
            outq = [singles.tile([128, NT * 12], f32, name=f"outq{q}")
                    for q in range(4)]
            dummy = singles.tile([1, 16], bf)

            nc.scalar.dma_start(bf_sb[:, :], bfin[:, :])
            nc.scalar.dma_start(f32_sb[:, :], f32in[:, :])
            nc.gpsimd.memset(fA[:, :, :], 0.0)
            nc.gpsimd.memset(fB[:, :, :], 0.0)

            # warm ACT's vector clock (1 wait per op) so conv relu-copies
            # only ever wait on PE.
            nc.scalar.copy(dummy[0:1, 0:1], bf_sb[0:1, 0:1])
            nc.scalar.copy(dummy[0:1, 1:2], f32_sb[0:1, 0:1])
            nc.scalar.copy(dummy[0:1, 2:3], fA[0:1, 0:1, 0:1])
            nc.scalar.copy(dummy[0:1, 3:4], fB[0:1, 0:1, 0:1])

            x_sb = bf_sb[:, 0:NR * WP].rearrange("p (r w) -> p r w", w=WP)
            cw_sb = bf_sb[:, NR * WP:NR * WP + 576].rearrange(
                "p (l t o) -> p l t o", t=9, o=16)
            w2p_sb = bf_sb[:, NR * WP + 576:].rearrange(
                "p (j c) -> p j c", c=432)
            w1_sb = f32_sb[0:3, 0:256]
            cb_sb = f32_sb[:, 256:260]
            b1_sb = f32_sb[:, 260:262]
            b2p_sb = f32_sb[0:1, 262:694]
            ones_sb = f32_sb[0:1, 694:822]
            shift_sb = f32_sb[:, 822:822 + NT * 12]

            # ---- conv chain ------------------------------------------
            # l: 0:x->fA  1:fA->fB  2:fB->fA  3:fA->fB, then fB->f4c
            fins = [x_sb, fA, fB, fA]
            fouts = [fA, fB, fA, fB]
            for l in range(4):
                K = 3 if l == 0 else 16
                fin, fout = fins[l], fouts[l]
                for ch in range(19):
                    r0 = 1 + 2 * ch
                    ps = cps.tile([128, 2, 256], f32, tag="convps")
                    for tap in range(9):
                        kh, kw = tap // 3, tap % 3
                        for n in range(4):
                            nc.tensor.matmul(
                                ps[32 * n:32 * n + 16, :, :],
                                cw_sb[32 * n:32 * n + K, l, tap, :],
                                fin[32 * n:32 * n + K,
                                    r0 + kh - 1:r0 + kh + 1,
                                    kw:kw + 256],
                                start=(tap == 0), stop=(tap == 8),
                                tile_position=(32 * n, 32 * n),
                            )
                    nc.scalar.activation(
                        fout[:, r0:r0 + 2, 1:257], ps[:, :, :],
                        mybir.ActivationFunctionType.Relu,
                        bias=cb_sb[:, l:l + 1], scale=1.0)

            # compact (32n+ci) -> contiguous 64 partitions for the xbar
            for n in range(4):
                nc.scalar.dma_start(
                    out=f4c[16 * n:16 * n + 16, :, :],
                    in_=fB[32 * n:32 * n + 16, :, :])

            # warm SP's clock on the 4 compaction DMAs (1 wait each)
            for n in range(4):
                nc.sync.dma_start(out=dummy[0:1, 4 + n:5 + n],
                                  in_=f4c[16 * n:16 * n + 1, 0:1, 0:1])

            # ---- im2col: shared row-transpose cache ------------------
            for r in range(34):
                for hf in range(2):
                    for kw in range(3):
                        nc.sync.dma_start_transpose(
                            out=fT2h[hf][:, (3 * r + kw) * 64:
                                         (3 * r + kw + 1) * 64],
                            in_=f4c[:, r + 3, 128 * hf + kw:
                                    128 * hf + kw + 128])
            fT2v = [t.rearrange("p (t x) -> p t x", x=64) for t in fT2h]

            # warm DVE's clock across the HWDGE sem lanes (recent
            # transposes cover the lanes; 1 wait per touch op).
            for k in range(9):
                r, hf, kw = 33 - (k // 6), (k // 3) % 2, k % 3
                nc.vector.tensor_copy(dummy[0:1, 13 + k % 3:14 + k % 3],
                                      fT2v[hf][0:1, 3 * r + kw, 0:1])

            # ---- per-q: h MLP, local weights, einsum -----------------
            mul, add = mybir.AluOpType.mult, mybir.AluOpType.add
            for q in range(4):
                for pc in range(PCH):
                    pos_t = pos_p.tile([3, 1024], f32, tag="pos")
                    nc.scalar.dma_start(
                        pos_t[:, :], post[q, :, pc * 1024:(pc + 1) * 1024])
                    hT = ht_p.tile([128, 2, 1024], bf, tag="ht")
                    for jh in range(2):
                        for hf in range(2):
                            hp = hps.tile([128, 512], f32, tag="hps")
                            nc.tensor.matmul(
                                hp[:, :],
                                w1_sb[:, jh * 128:(jh + 1) * 128],
                                pos_t[:, hf * 512:(hf + 1) * 512],
                                start=True, stop=True)
                            nc.scalar.activation(
                                hT[:, jh, hf * 512:(hf + 1) * 512], hp[:, :],
                                mybir.ActivationFunctionType.Relu,
                                bias=b1_sb[:, jh:jh + 1], scale=1.0)
                    for tl in range(8):
                        t = pc * 8 + tl
                        r0, hf = t // 2, t % 2
                        lwp = lps.tile([128, 3, 9, 16], f32, tag="lwp")
                        for jh in range(2):
                            nc.tensor.matmul(
                                lwp[:, :, :, :],
                                hT[:, jh, tl * 128:(tl + 1) * 128],
                                w2p_sb[:, jh, :],
                                start=(jh == 0), stop=False)
                        nc.tensor.matmul(
                            lwp[:, :, :, :], ones_sb[:, :], b2p_sb[:, :],
                            start=False, stop=True)
                        lws = lws_p.tile([128, 3, 9, 16], bf, tag="lws")
                        nc.scalar.activation(
                            lws[:, :, :, :], lwp[:, :, :, :],
                            mybir.ActivationFunctionType.Copy)
                        for n in range(4):
                            for c in range(3):
                                scr = scr_p.tile([128, 9, 16], bf,
                                                 tag="scr")
                                nc.vector.scalar_tensor_tensor(
                                    out=scr[:, :, :],
                                    in0=fT2v[hf][:, 3 * r0:3 * r0 + 9,
                                                 n * 16:(n + 1) * 16],
                                    scalar=1.0,
                                    in1=lws[:, c, :, :],
                                    op0=mul, op1=mul,
                                    accum_out=outq[q][:, (n * 3 + c) * NT + t:
                                                      (n * 3 + c) * NT + t + 1])
                # add_mean: +255*RGB_MEAN[c] to every output element
                nc.vector.tensor_add(outq[q][:, :], outq[q][:, :],
                                     shift_sb)

            # ---- writeback -------------------------------------------
            for q in range(4):
                si, sj = q // 2, q % 2
                dstv = outd.rearrange(
                    "n c (h2 s1) (w1 w2 s2) -> n c s1 s2 w2 h2 w1",
                    s1=2, w1=2, s2=2)
                for n in range(4):
                    for c in range(3):
                        nci = n * 3 + c
                        src = outq[q][:, nci * NT:(nci + 1) * NT].rearrange(
                            "p (t2 t1) -> p t2 t1", t1=2)
                        for w1 in range(2):
                            nc.gpsimd.dma_start(
                                out=dstv[n, c, si, sj, :, :, w1],
                                in_=src[:, :, w1])
    _legalize_waits(nc)
    return nc


def _get_nc():
    global _NC
    if _NC is None:
        _NC = _build_program()
    return _NC


def _prep_inputs(x, pos_mat, c0w, c0b, c1w, c1b, c2w, c2b, c3w, c3b,
                 w1, b1, w2, b2):
    """Host-side packing of per-core input dicts."""
    x = np.asarray(x, np.float32)
    pos = np.asarray(pos_mat, np.float32).reshape(-1, 3)

    # conv weights: cw[32n+ci, l, kh*3+kw, co]
    cwp = np.zeros((128, 4, 9, 16), np.float32)
    cbp = np.zeros((128, 4), np.float32)
    for l, (wl, bl) in enumerate(((c0w, c0b), (c1w, c1b),
                                  (c2w, c2b), (c3w, c3b))):
        wl = np.asarray(wl, np.float32)          # (co, ci, 3, 3)
        K = wl.shape[1]
        t = wl.transpose(1, 2, 3, 0).reshape(K, 9, 16)   # (ci, tap, co)
        for n in range(4):
            cwp[32 * n:32 * n + K, l] = t
            cbp[32 * n:32 * n + 16, l] = np.asarray(bl, np.float32)

    w1 = np.asarray(w1, np.float32)              # (3, 256)
    b1p = np.asarray(b1, np.float32).reshape(2, 128).T.copy()  # [j, jh]

    # w2 columns: orig (s=ci*9+tap, c) -> permuted (c, tap, ci)
    w2 = np.asarray(w2, np.float32).reshape(256, 16, 9, 3)     # j, ci, tap, c
    w2pm = w2.transpose(0, 3, 2, 1).reshape(256, 432)          # j,(c,tap,ci)
    w2pk = w2pm.reshape(2, 128, 432).astype(BF16)              # [jh, j, 432]
    w2pk = np.ascontiguousarray(w2pk.transpose(1, 0, 2))       # [j, jh, 432]
    b2 = np.asarray(b2, np.float32).reshape(16, 9, 3)
    b2pk = b2.transpose(2, 1, 0).reshape(1, 432)

    # pos rows ordered (h, si, w, sj); per-core chunk -> (q, 3, NPIX)
    posr = pos.reshape(Himg, 2, Wimg, 2, 3)

    # f32 pack: [w1 | cb | b1c | b2p | ones | mean-shift]
    f32pk = np.zeros((128, 822 + NT * 12), np.float32)
    f32pk[0:3, 0:256] = w1
    f32pk[:, 256:260] = cbp
    f32pk[:, 260:262] = b1p
    f32pk[0, 262:694] = b2pk[0]
    f32pk[0, 694:822] = 1.0
    shift = np.zeros(NT * 12, np.float32)
    for n in range(4):
        for c in range(3):
            shift[(n * 3 + c) * NT:(n * 3 + c + 1) * NT] = \
                RGB_RANGE * RGB_MEAN[c]
    f32pk[:, 822:] = shift

    in_maps = []
    for core in range(NCORES):
        h0 = core * ROWS
        xh = np.zeros((128, NR, WP), np.float32)
        lo, hi = h0 - HALO, h0 + ROWS + HALO
        slo, shi = max(lo, 0), min(hi, Himg)
        for n in range(4):
            xh[32 * n:32 * n + 3, slo - lo:shi - lo, 1:257] = \
                x[n, :, slo:shi, :]
        bfpk = np.concatenate(
            [xh.reshape(128, -1), cwp.reshape(128, -1),
             w2pk.reshape(128, -1).astype(np.float32)], axis=1)
        pc = posr[h0:h0 + ROWS].transpose(1, 3, 4, 0, 2)  # si,sj,3,h,w
        pc = pc.reshape(2, 2, 3, NPIX).reshape(4, 3, NPIX)
        in_maps.append({
            "bfin": bfpk.astype(BF16),
            "f32in": f32pk,
            "post": np.ascontiguousarray(pc),
        })
    return in_maps


LAST_RESULTS = None
TRACE = False


def kernel(**inputs):
    global LAST_RESULTS
    nc = _get_nc()
    in_maps = _prep_inputs(**inputs)
    res = run_bass_kernel_spmd(nc, in_maps, core_ids=list(range(NCORES)),
                               trace=TRACE)
    LAST_RESULTS = res
    out = np.concatenate([res.results[i]["out"] for i in range(NCORES)],
                         axis=2)
    return out.astype(np.float32)



# revision 8
# speedup vs baseline: 2.1071x; 2.1071x over previous
"""MetaQuickSR Trainium2 kernel (8-core SPMD, row-sharded), v2.

Sharding: H=256 output-feature rows split 32/core (+4-row conv halo).
Each core: 4-layer CNN (block-diagonal image batching on PE) -> PE-based
im2col row transposes -> bf16 Pos2Weight MLP -> per-pixel locally-
connected einsum split across DVE+Pool -> transpose/interleave writeback
with contiguous output DMAs.  No cross-core communication.
"""

import numpy as np
import ml_dtypes

import concourse.bass as bass
import concourse.mybir as mybir
from concourse.tile import TileContext
from concourse.bass_utils import run_bass_kernel_spmd
from concourse.masks import make_identity

BF16 = ml_dtypes.bfloat16

NCORES = 8
N, CI, Himg, Wimg, S = 4, 16, 256, 256, 2
ROWS = Himg // NCORES          # 32 output-feature rows per core
HALO = 4
NR = ROWS + 2 * HALO           # 40 buffered rows
WP = Wimg + 2                  # 258 zero-padded width
NPIX = ROWS * Wimg             # 8192 einsum pixels per core
NT = NPIX // 128               # 64 pixel tiles per q plane
RGB_MEAN = (0.4488, 0.4371, 0.404)
RGB_RANGE = 255.0

XW = NR * WP + 256             # dense x (12 parts) + w1 rows 0-2
WW = 4 * 9 * 128 + 2 * 432 + 432   # cwB + w2p + b2p(row0)

# (n,c) pairs handled by Pool (gpsimd); the rest on DVE.
# (TensorScalarPtr fails the Pool ISA engine check -> keep empty.)
POOL_NC = frozenset()

_NC = None


def _legalize_waits(nc, lim=1):
    """This walrus build accepts only one sync-wait per instruction; move
    surplus waits onto same-engine NoOps inserted just before."""
    cnt = 0
    for f in nc.m.functions:
        for bb in f.blocks:
            new = []
            for inst in bb.instructions:
                si = inst.sync_info
                if si is not None and si.on_wait is not None \
                        and len(si.on_wait) > lim:
                    waits = list(si.on_wait)
                    excess, keep = waits[:-lim], waits[-lim:]
                    for w in excess:
                        cnt += 1
                        nop = mybir.InstNoOp(
                            name=f"I-lw{cnt}", opcode="NoOp",
                            engine=inst.engine, debug=inst.debug,
                            ins=[], outs=[],
                            sync_info=mybir.SyncInfo(on_wait=[w],
                                                     on_update=[]))
                        new.append(nop)
                        nc.inst_map[nop.name] = nop
                    inst.sync_info = mybir.SyncInfo(
                        on_wait=keep, on_update=list(si.on_update or []))
                new.append(inst)
            bb.instructions = new
    return cnt


def _build_program():
    nc = bass.Bass(trn_type="TRN2")
    f32 = mybir.dt.float32
    bf = mybir.dt.bfloat16

    xin = nc.dram_tensor("xin", [12, XW], bf, kind="ExternalInput")
    win = nc.dram_tensor("win", [128, WW], bf, kind="ExternalInput")
    fin32 = nc.dram_tensor("fin32", [128, 12], f32, kind="ExternalInput")
    post = nc.dram_tensor("post", [4, 3, NPIX], bf, kind="ExternalInput")
    outd = nc.dram_tensor("out", [4, 3, 2 * ROWS, 2 * Wimg], f32,
                          kind="ExternalOutput")

    mul = mybir.AluOpType.mult

    with TileContext(nc) as tc:
        with (
            tc.tile_pool(name="singles", bufs=1) as singles,
            tc.tile_pool(name="pos_p", bufs=2) as pos_p,
            tc.tile_pool(name="ht_p", bufs=2) as ht_p,
            tc.tile_pool(name="lws_p", bufs=3) as lws_p,
            tc.tile_pool(name="scr_d", bufs=2) as scr_d,
            tc.tile_pool(name="scr_g", bufs=2) as scr_g,
        ):
            # ---- resident tiles --------------------------------------
            xw_sb = singles.tile([12, XW], bf)
            win_sb = singles.tile([128, WW], bf)
            f32_sb = singles.tile([128, 12], f32)
            fA = singles.tile([128, NR, WP], bf)
            fB = singles.tile([128, NR, WP], bf)
            f4c = singles.tile([64, NR, WP], bf)
            # fT2h[hf][pix, (row 34, kw 3, (n,ci) 64)]
            fT2h = [singles.tile([128, 34 * 3 * 64], bf, name=f"fT2h{h}")
                    for h in range(2)]
            outq = [singles.tile([128, 768], f32, name=f"outq{q}")
                    for q in range(4)]
            staged = [singles.tile([128, 6, 256], f32, name=f"stg{s}")
                      for s in range(2)]
            ones_sb = singles.tile([1, 128], bf)
            idbf = singles.tile([64, 64], bf)
            idf32 = singles.tile([128, 128], f32)
            dummy = singles.tile([1, 16], bf)

            nc.sync.dma_start(xw_sb[:, :], xin[:, :])
            nc.scalar.dma_start(win_sb[:, :], win[:, :])
            nc.scalar.dma_start(f32_sb[:, :], fin32[:, :])
            nc.gpsimd.memset(fA[:, :, :], 0.0)
            nc.gpsimd.memset(fB[:, :, :], 0.0)
            nc.gpsimd.memset(ones_sb[:, :], 1.0)
            nc.gpsimd.memset(staged[0][:, :, :], 0.0)
            nc.gpsimd.memset(staged[1][:, :, :], 0.0)
            make_identity(nc, idbf)
            make_identity(nc, idf32)

            xv = xw_sb[:, 0:NR * WP].rearrange("p (r w) -> p r w", w=WP)
            w1v = xw_sb[0:3, NR * WP:NR * WP + 256]
            cw = win_sb[:, 0:4608].rearrange("p (l t o) -> p l t o",
                                             t=9, o=128)
            w2pv = win_sb[:, 4608:4608 + 864].rearrange(
                "p (j c) -> p j c", c=432)
            b2pv = win_sb[0:1, 5472:5904]
            cb = f32_sb[:, 0:4]
            b1c = f32_sb[:, 4:6]
            shiftv = f32_sb[:, 6:12]

            # warm ACT's vector clock (1 wait per op) so conv relu-copies
            # only ever wait on PE.
            nc.scalar.copy(dummy[0:1, 0:1], xw_sb[0:1, 0:1])
            nc.scalar.copy(dummy[0:1, 1:2], win_sb[0:1, 0:1])
            nc.scalar.copy(dummy[0:1, 2:3], fA[0:1, 0:1, 0:1])
            nc.scalar.copy(dummy[0:1, 3:4], fB[0:1, 0:1, 0:1])

            # ---- conv chain + interleaved im2col ---------------------
            # l: 0:x->fA  1:fA->fB  2:fB->fA  3:fA->fB
            fins = [xv, fA, fB, fA]
            fouts = [fA, fB, fA, fB]

            def compact_rows(r0, r1):
                for n in range(4):
                    nc.sync.dma_start(
                        out=f4c[16 * n:16 * n + 16, r0:r1, :],
                        in_=fB[32 * n:32 * n + 16, r0:r1, :])

            def transpose_rows(rr):
                # r in fT2 coords (f4 row = r+3)
                for r in rr:
                    for hf in range(2):
                        tp = tps.tile([128, 3, 64], bf, tag="tps")
                        for kw in range(3):
                            nc.tensor.transpose(
                                tp[:, kw, :],
                                f4c[:, r + 3, 128 * hf + kw:
                                    128 * hf + kw + 128],
                                idbf[:, :])
                        nc.vector.tensor_copy(
                            fT2h[hf][:, 3 * r * 64:3 * (r + 1) * 64],
                            tp[:, :, :])

            with tc.tile_pool(name="cps", bufs=2, space="PSUM") as cps, \
                 tc.tile_pool(name="tps", bufs=3, space="PSUM") as tps:
                for l in range(4):
                    fin, fout = fins[l], fouts[l]
                    for ch in range(19):
                        r0 = 1 + 2 * ch
                        ps = cps.tile([128, 2, 256], f32, tag="convps")
                        for tap in range(9):
                            kh, kw = tap // 3, tap % 3
                            if l == 0:
                                lhsT = cw[0:12, 0, tap, :]
                                rhs = fin[0:12, r0 + kh - 1:r0 + kh + 1,
                                          kw:kw + 256]
                            else:
                                lhsT = cw[:, l, tap, :]
                                rhs = fin[:, r0 + kh - 1:r0 + kh + 1,
                                          kw:kw + 256]
                            nc.tensor.matmul(
                                ps[:, :, :], lhsT, rhs,
                                start=(tap == 0), stop=(tap == 8))
                        nc.scalar.activation(
                            fout[:, r0:r0 + 2, 1:257], ps[:, :, :],
                            mybir.ActivationFunctionType.Relu,
                            bias=cb[:, l:l + 1], scale=1.0)
                        # layer 3: compact + transpose finished row groups
                        if l == 3:
                            if ch == 7:
                                compact_rows(3, 13)      # f4 rows 3-12
                            elif ch == 9:
                                transpose_rows(range(0, 10))
                            elif ch == 12:
                                compact_rows(13, 23)
                            elif ch == 14:
                                transpose_rows(range(10, 20))
                            elif ch == 17:
                                compact_rows(23, 33)
                    if l == 3:
                        compact_rows(33, 37)
                        transpose_rows(range(20, 34))

            # ---- per-q: h MLP, local weights, einsum, writeback ------
            with tc.tile_pool(name="hps", bufs=2, space="PSUM") as hps, \
                 tc.tile_pool(name="lps", bufs=2, space="PSUM") as lps, \
                 tc.tile_pool(name="wps", bufs=2, space="PSUM") as wps:
                fT2v = [t.rearrange("p (t x) -> p t x", x=64)
                        for t in fT2h]
                for q in range(4):
                    si, sj = q // 2, q % 2
                    for pc in range(8):
                        pos_t = pos_p.tile([3, 1024], bf, tag="pos")
                        nc.scalar.dma_start(
                            pos_t[:, :],
                            post[q, :, pc * 1024:(pc + 1) * 1024])
                        hT = ht_p.tile([128, 2, 1024], bf, tag="ht")
                        for jh in range(2):
                            for hf2 in range(2):
                                hp = hps.tile([128, 512], f32, tag="hps")
                                nc.tensor.matmul(
                                    hp[:, :],
                                    w1v[:, jh * 128:(jh + 1) * 128],
                                    pos_t[:, hf2 * 512:(hf2 + 1) * 512],
                                    start=True, stop=True)
                                nc.scalar.activation(
                                    hT[:, jh, hf2 * 512:(hf2 + 1) * 512],
                                    hp[:, :],
                                    mybir.ActivationFunctionType.Relu,
                                    bias=b1c[:, jh:jh + 1], scale=1.0)
                        for tl in range(8):
                            t = pc * 8 + tl
                            r0, hf = t // 2, t % 2
                            lwp = lps.tile([128, 3, 9, 16], f32,
                                           tag="lwp")
                            for jh in range(2):
                                nc.tensor.matmul(
                                    lwp[:, :, :, :],
                                    hT[:, jh, tl * 128:(tl + 1) * 128],
                                    w2pv[:, jh, :],
                                    start=(jh == 0), stop=False)
                            nc.tensor.matmul(
                                lwp[:, :, :, :], ones_sb[:, :], b2pv,
                                start=False, stop=True)
                            lws = lws_p.tile([128, 3, 9, 16], bf,
                                             tag="lws")
                            nc.scalar.activation(
                                lws[:, :, :, :], lwp[:, :, :, :],
                                mybir.ActivationFunctionType.Copy)
                            for n in range(4):
                                for c in range(3):
                                    nci = n * 3 + c
                                    if nci in POOL_NC:
                                        eng, pool = nc.gpsimd, scr_g
                                    else:
                                        eng, pool = nc.vector, scr_d
                                    scr = pool.tile([128, 9, 16], bf,
                                                    tag="scr")
                                    eng.scalar_tensor_tensor(
                                        out=scr[:, :, :],
                                        in0=fT2v[hf][:, 3 * r0:3 * r0 + 9,
                                                     16 * n:16 * n + 16],
                                        scalar=1.0,
                                        in1=lws[:, c, :, :],
                                        op0=mul, op1=mul,
                                        accum_out=outq[q][:,
                                                          nci * 64 + t:
                                                          nci * 64 + t + 1])
                    # writeback: transpose + sj-interleave (+mean shift)
                    for j in range(6):
                        tq = wps.tile([128, 128], f32, tag="wps")
                        nc.tensor.transpose(
                            tq[:, :], outq[q][:, 128 * j:128 * (j + 1)],
                            idf32[:, :])
                        nc.scalar.activation(
                            staged[si].rearrange(
                                "p j (w s) -> p j w s", s=2)[:, j, :, sj],
                            tq[:, :],
                            mybir.ActivationFunctionType.Identity,
                            bias=shiftv[:, j:j + 1], scale=1.0)
                    if sj == 1:
                        # src partitions walk (a=nci_lo, r, h)-major then w;
                        # dst dims [a, r, h, w] match that element order.
                        dstv = outd.rearrange(
                            "n c (r s) (h w) -> (n c) s r h w",
                            s=2, h=2)
                        for j in range(6):
                            nc.sync.dma_start(
                                out=dstv[2 * j:2 * j + 2, si, :, :, :],
                                in_=staged[si][:, j, :])
    _legalize_waits(nc)
    return nc


def _get_nc():
    global _NC
    if _NC is None:
        _NC = _build_program()
    return _NC


def _prep_inputs(x, pos_mat, c0w, c0b, c1w, c1b, c2w, c2b, c3w, c3b,
                 w1, b1, w2, b2):
    """Host-side packing of per-core input dicts."""
    x = np.asarray(x, np.float32)
    pos = np.asarray(pos_mat, np.float32).reshape(-1, 3)

    # block-diagonal conv weights cwB[p, l, tap, 32n+co]
    cwB = np.zeros((128, 4, 9, 128), np.float32)
    cbp = np.zeros((128, 4), np.float32)
    for l, (wl, bl) in enumerate(((c0w, c0b), (c1w, c1b),
                                  (c2w, c2b), (c3w, c3b))):
        wl = np.asarray(wl, np.float32)          # (co, ci, 3, 3)
        K = wl.shape[1]
        t = wl.transpose(1, 2, 3, 0).reshape(K, 9, 16)   # (ci, tap, co)
        for n in range(4):
            if l == 0:
                cwB[3 * n:3 * n + K, l, :, 32 * n:32 * n + 16] = t
            else:
                cwB[32 * n:32 * n + K, l, :, 32 * n:32 * n + 16] = t
            cbp[32 * n:32 * n + 16, l] = np.asarray(bl, np.float32)

    w1 = np.asarray(w1, np.float32)              # (3, 256)
    b1p = np.asarray(b1, np.float32).reshape(2, 128).T.copy()  # [j, jh]

    # w2 columns: orig (s=ci*9+tap, c) -> permuted (c, tap, ci)
    w2 = np.asarray(w2, np.float32).reshape(256, 16, 9, 3)     # j,ci,tap,c
    w2pm = w2.transpose(0, 3, 2, 1).reshape(256, 432)          # j,(c,t,ci)
    w2pk = w2pm.reshape(2, 128, 432)                           # [jh,j,432]
    w2pk = np.ascontiguousarray(w2pk.transpose(1, 0, 2))       # [j,jh,432]
    b2 = np.asarray(b2, np.float32).reshape(16, 9, 3)
    b2pk = b2.transpose(2, 1, 0).reshape(432)

    # win pack: [cwB | w2p | b2p]
    winpk = np.zeros((128, WW), np.float32)
    winpk[:, 0:4608] = cwB.reshape(128, 4608)
    winpk[:, 4608:5472] = w2pk.reshape(128, 864)
    winpk[0, 5472:5904] = b2pk

    # fin32: [cb | b1c | shift(j)]
    f32pk = np.zeros((128, 12), np.float32)
    f32pk[:, 0:4] = cbp
    f32pk[:, 4:6] = b1p
    for j in range(6):
        for p in range(128):
            nci = 2 * j + (1 if p >= 64 else 0)
            f32pk[p, 6 + j] = RGB_RANGE * RGB_MEAN[nci % 3]

    # pos rows ordered (h, si, w, sj); per-core chunk -> (q, 3, NPIX)
    posr = pos.reshape(Himg, 2, Wimg, 2, 3)

    in_maps = []
    for core in range(NCORES):
        h0 = core * ROWS
        xh = np.zeros((12, NR, WP), np.float32)
        lo, hi = h0 - HALO, h0 + ROWS + HALO
        slo, shi = max(lo, 0), min(hi, Himg)
        for n in range(4):
            xh[3 * n:3 * n + 3, slo - lo:shi - lo, 1:257] = \
                x[n, :, slo:shi, :]
        xpack = np.zeros((12, XW), np.float32)
        xpack[:, :NR * WP] = xh.reshape(12, -1)
        xpack[0:3, NR * WP:] = w1
        pc = posr[h0:h0 + ROWS].transpose(1, 3, 4, 0, 2)  # si,sj,3,h,w
        pc = pc.reshape(4, 3, NPIX)
        in_maps.append({
            "xin": xpack.astype(BF16),
            "win": winpk.astype(BF16),
            "fin32": f32pk,
            "post": np.ascontiguousarray(pc).astype(BF16),
        })
    return in_maps


LAST_RESULTS = None
TRACE = False


def kernel(**inputs):
    global LAST_RESULTS
    nc = _get_nc()
    in_maps = _prep_inputs(**inputs)
    res = run_bass_kernel_spmd(nc, in_maps, core_ids=list(range(NCORES)),
                               trace=TRACE)
    LAST_RESULTS = res
    out = np.concatenate([res.results[i]["out"] for i in range(NCORES)],
                         axis=2)
    return out.astype(np.float32)
